# revision 17
# baseline (speedup 1.0000x reference)
"""Tensor-parallel causal attention block (qkv proj + RoPE + attention + out proj)
for Trainium2, sharded over 8 NeuronCores by attention head (2 heads/core).

Contract: kernel(**inputs) takes the FULL inputs (x [1,2048,1024] f32,
w_in [3072,1024] f32, w_out [1024,1024] f32, is_causal scalar) and returns the
FULL output [1,2048,1024] f32.

Per-core layout strategy (everything kept transposed, [feature, seq], so no
on-device transposes of activations are ever needed):
  - host pre-transposes x -> xT [1024,2048] and the weight shards, all bf16
    (matmul operands bf16, f32 PSUM accumulate; rel-err budget is 2e-2)
  - qkvT = w_shard @ xT  ->  [384, 2048] (Q.T | K.T | V.T rows, 2 heads packed)
  - RoPE applied in [hd, s] layout via a constant rotation matmul + elementwise
  - scores computed transposed: S.T[k, q] = K @ Q.T  (softmax dim = partitions)
  - probs (unnormalized exp) hit PV directly:  ctx.T = V_aug.T @ P.T, where
    V_aug carries a ones column so row 64 of the PV output is the softmax
    denominator; 1/denom = exp(-ln denom) on the scalar engine (ln+exp share
    an act table; DVE reciprocal on a 1-partition row costs 3.3us)
  - out partial = ctx @ w_out_shard.T with both heads packed into one
    128-contraction matmul; host sums the 8 partials (the TP all-reduce is a
    plain numpy sum of disjoint-head partials).

Schedule: the PE HAM clock-gate re-throttles to 1.2 GHz whenever the PE has
idle in a ~3.4us window, so the whole schedule is built to keep the PE matmul
stream dense: DMA loads are issued in compute-priority order, the input
projection is chopped into per-512-column groups interleaved into the
attention chunks as PE filler work, V transposes go through the DMA xbar
(dma_start_transpose) instead of the PE, and the softmax exp runs as a single
wide ACT op per k-tile so the scalar engine can keep pace.
"""
import sys

sys.path.insert(0, '/opt/trn_rl_repo')

from contextlib import ExitStack

import ml_dtypes
import numpy as np

import concourse.bass as bass
from concourse import mybir, tile
from concourse.bass_utils import run_bass_kernel_spmd

B, S, D, H = 1, 2048, 1024, 16
HD = D // H            # 64
NCORES = 8
HPC = H // NCORES      # heads per core = 2
EPC = HPC * HD         # features per core = 128
ROPE_BASE = 10000.0

F32 = mybir.dt.float32
BF16 = mybir.dt.bfloat16
BF16NP = ml_dtypes.bfloat16

QC = 512               # q-chunk width (one PSUM bank of fp32)
NQC = S // QC          # 4 q-chunks
NST = S // 128         # 16 s-tiles / k-tiles
ND = D // 128          # 8 contraction tiles for the input projection
EXP = mybir.ActivationFunctionType.Exp
LN = mybir.ActivationFunctionType.Ln


def _split_multi_waits(nc, max_waits=1):
    """This container's walrus build accepts at most one embedded sync wait per
    instruction; move extra waits onto preceding same-engine NoOps."""
    n_split = 0
    for fn in nc.m.functions:
        for blk in fn.blocks:
            new_insts = []
            for inst in blk.instructions:
                si = inst.sync_info
                waits = list(si.on_wait) if (si and si.on_wait) else []
                if len(waits) > max_waits and inst.engine is not None:
                    for w in waits[max_waits:]:
                        nop = mybir.InstNoOp(
                            name=f"{inst.name}_wn{n_split}", ins=[], outs=[])
                        n_split += 1
                        nop.engine = inst.engine
                        nop.sync_info = mybir.SyncInfo(on_wait=[w], on_update=[])
                        nc.register_instruction(nop, overwrite=True)
                        new_insts.append(nop)
                    si.on_wait = waits[:max_waits]
                new_insts.append(inst)
            blk.instructions[:] = new_insts
    return n_split


def _host_constants():
    inv_freq = 1.0 / (ROPE_BASE ** (np.arange(0, HD, 2, dtype=np.float64) / HD))
    t = np.arange(S, dtype=np.float64)
    freqs = np.outer(inv_freq, t)                    # [32, S]  ([hd, s] layout)
    emb = np.concatenate([freqs, freqs], axis=0)     # [64, S]
    cosT = np.cos(emb).astype(BF16NP)
    sinT = np.sin(emb).astype(BF16NP)
    cos2 = np.tile(cosT, (2, 1))                     # [128, S] (2 heads packed)
    sin2 = np.tile(sinT, (2, 1))
    # rotate_half as a matrix: (R q)[i] = -q[i+32] (i<32), q[i-32] (i>=32)
    R = np.zeros((HD, HD), dtype=np.float32)
    for i in range(HD // 2):
        R[i, i + HD // 2] = -1.0
        R[i + HD // 2, i] = 1.0
    R2 = np.zeros((128, 128), dtype=np.float32)
    R2[0:64, 0:64] = R
    R2[64:128, 64:128] = R
    rotT = np.ascontiguousarray(R2.T).astype(BF16NP)
    # upper-triangular (k<=q) mask for the diagonal 128x128 blocks of S.T[k,q],
    # duplicated side by side so one 3D DVE op masks both heads at once
    tri1 = np.triu(np.ones((128, 128), dtype=np.float32))
    tri2 = np.concatenate([tri1, tri1], axis=1).astype(BF16NP)   # [128, 256]
    ident = np.eye(128, dtype=np.float32).astype(BF16NP)
    return cos2, sin2, rotT, tri2, ident


def _build_program(causal: bool):
    nc = bass.Bass()
    xT_d = nc.dram_tensor("xT", [D, S], BF16, kind="ExternalInput")
    winT_d = nc.dram_tensor("winT", [D, 3 * EPC], BF16, kind="ExternalInput")
    woT_d = nc.dram_tensor("woT", [EPC, D], BF16, kind="ExternalInput")
    pout_d = nc.dram_tensor("pout", [S, D], BF16, kind="ExternalOutput")

    cos2_np, sin2_np, rotT_np, tri_np, ident_np = _host_constants()
    cos2_d = nc.inline_tensor(cos2_np, name="cos2")
    sin2_d = nc.inline_tensor(sin2_np, name="sin2")
    rotT_d = nc.dram_tensor("rotT", [128, 128], BF16, kind="ExternalInput")
    tri_d = nc.dram_tensor("tri", [128, 2 * 128], BF16, kind="ExternalInput")
    ident_d = nc.dram_tensor("ident", [128, 128], BF16, kind="ExternalInput")

    with tile.TileContext(nc) as tc, ExitStack() as ctx:
        sb = ctx.enter_context(tc.tile_pool(name="sb", bufs=1))
        wk0 = ctx.enter_context(tc.tile_pool(name="wk0", bufs=1))

        # ---- persistent SBUF tensors -----------------------------------
        winT = sb.tile([128, ND, 3 * EPC], BF16, name="winT")
        woT = sb.tile([EPC, D], BF16, name="woT")
        cos2 = sb.tile([128, S], BF16, name="cos2")
        sin2 = sb.tile([128, S], BF16, name="sin2")
        rot = sb.tile([128, 128], BF16, name="rot")
        tri = sb.tile([128, 2, 128], BF16, name="tri")
        ident = sb.tile([128, 128], BF16, name="ident")
        xtall = sb.tile([128, ND, S], BF16, name="xtall")

        onesf = sb.tile([128, HD], BF16, name="onesf")
        nc.vector.memset(onesf[:], 1.0)
        qraw = sb.tile([128, S], BF16, name="qraw")
        kraw = sb.tile([128, S], BF16, name="kraw")
        vtr = sb.tile([128, S], BF16, name="vtr")
        qrot = sb.tile([128, S], BF16, name="qrot")
        krot = sb.tile([128, S], BF16, name="krot")
        vnat = sb.tile([128, NST * 130], BF16, name="vnat")
        nc.vector.memset(vnat[:], 1.0)
        heatout = sb.tile([1, 1], F32, name="heatout")
        ctxt = sb.tile([EPC, S], BF16, name="ctxt")

        # ---- DMA loads in compute-priority order -----------------------
        # Few, large, strided DMAs: each dma_start costs ~600ns of issue
        # time on its HWDGE queue, so 36 small loads would be issue-bound.
        # Critical stream (winT + xT group 0) on Sync; constants on Scalar.
        def xt_load(g, dlo, dhi):
            nc.sync.dma_start(
                xtall[:, dlo:dhi, g * QC:(g + 1) * QC],
                xT_d[dlo * 128:dhi * 128, g * QC:(g + 1) * QC]
                .rearrange("(d p) c -> p d c", d=dhi - dlo))

        nc.sync.dma_start(winT[:, 0:4, :],
                          winT_d[0:512, :].rearrange("(d p) c -> p d c", d=4))
        xt_load(0, 0, 4)
        nc.sync.dma_start(winT[:, 4:ND, :],
                          winT_d[512:1024, :].rearrange("(d p) c -> p d c",
                                                        d=4))
        xt_load(0, 4, ND)
        nc.sync.dma_start(cos2[:, 0:2 * QC], cos2_d[:, 0:2 * QC])
        nc.sync.dma_start(sin2[:, 0:2 * QC], sin2_d[:, 0:2 * QC])
        for g in range(1, 4):
            xt_load(g, 0, ND)
        nc.scalar.dma_start(rot[:], rotT_d[:, :])
        nc.scalar.dma_start(tri[:], tri_d[:, :])
        nc.scalar.dma_start(ident[:], ident_d[:, :])
        nc.scalar.dma_start(woT[:], woT_d[:, :])
        nc.scalar.dma_start(cos2[:, 2 * QC:S], cos2_d[:, 2 * QC:S])
        nc.scalar.dma_start(sin2[:, 2 * QC:S], sin2_d[:, 2 * QC:S])

        def rope_chunk(c0, pfn):
            for (raw, out) in ((qraw, qrot), (kraw, krot)):
                t1 = wk0.tile([128, QC], BF16, tag="t1", bufs=3, name="t1")
                nc.gpsimd.tensor_mul(t1[:], raw[:, c0:c0 + QC],
                                     cos2[:, c0:c0 + QC])
                rp = pfn()
                nc.tensor.matmul(rp[:, 0:QC], rot[:], raw[:, c0:c0 + QC],
                                 start=True, stop=True)
                t2 = wk0.tile([128, QC], BF16, tag="t2", bufs=3, name="t2")
                nc.vector.tensor_mul(t2[:], rp[:, 0:QC], sin2[:, c0:c0 + QC])
                nc.vector.tensor_add(out[:, c0:c0 + QC], t1[:], t2[:])

        def vt_tile(j, vfn):
            vp = vfn()
            nc.tensor.transpose(vp[:, 0:128], vtr[:, j * 128:(j + 1) * 128],
                                ident[:])
            nc.vector.tensor_copy(vnat[:, j * 130:j * 130 + 64], vp[:, 0:64])
            nc.vector.tensor_copy(vnat[:, j * 130 + 65:j * 130 + 129],
                                  vp[:, 64:128])

        # ========== Stage A: QKV/rope/vt for cols [0, 512) ==============
        with tc.tile_pool(name="psA", bufs=1, space="PSUM") as psA:
            accs = [psA.tile([128, QC], F32, tag="acc", bufs=4, name="acc")
                    for _ in range(3)]
            for d in range(ND):
                for et in range(3):
                    lw = winT[:, d, et * 128:(et + 1) * 128]
                    nc.tensor.matmul(accs[et][:], lw, xtall[:, d, 0:QC],
                                     start=(d == 0), stop=(d == ND - 1))
            # ACT is otherwise idle here; rope is gated on these evictions
            nc.scalar.copy(qraw[:, 0:QC], accs[0][:])
            nc.scalar.copy(kraw[:, 0:QC], accs[1][:])
            nc.scalar.copy(vtr[:, 0:QC], accs[2][:])

            def pa_tile():
                return psA.tile([128, QC], F32, tag="acc", bufs=4, name="rp")

            def pa_vt():
                return psA.tile([128, 128], BF16, tag="acc", bufs=4,
                                name="vp")
            rope_chunk(0, pa_tile)
            for j in range(4):
                vt_tile(j, pa_vt)

        # ========== attention + interleaved filler work =================
        with tc.tile_pool(name="psB", bufs=1, space="PSUM") as psB, \
             tc.tile_pool(name="wkb", bufs=3) as wkb:

            def op_tile():
                return psB.tile([128, QC], F32, tag="op", bufs=2, name="op")

            # ---- filler thunks: QKV group for cols [g*512, (g+1)*512) ----
            acc_h = {}

            def qkv_thunk(g, et, lohi):
                def f():
                    dlo, dhi = lohi
                    c0 = g * QC
                    if dlo == 0:
                        acc_h[(g, et)] = op_tile()
                    acc = acc_h[(g, et)]
                    for d in range(dlo, dhi):
                        lw = winT[:, d, et * 128:(et + 1) * 128]
                        nc.tensor.matmul(acc[:], lw, xtall[:, d, c0:c0 + QC],
                                         start=(d == 0), stop=(d == ND - 1))
                    if dhi == ND:
                        dst = (qraw, kraw, vtr)[et]
                        nc.vector.tensor_copy(dst[:, c0:c0 + QC], acc[:])
                        del acc_h[(g, et)]
                return f

            def group_fillers(g):
                fs = []
                for et in (0, 1):
                    fs.append(qkv_thunk(g, et, (0, 4)))
                    fs.append(qkv_thunk(g, et, (4, ND)))
                fs.append(lambda g=g: rope_chunk(g * QC, op_tile))
                fs.append(qkv_thunk(g, 2, (0, 4)))
                fs.append(qkv_thunk(g, 2, (4, ND)))
                def op_vt():
                    return psB.tile([128, 128], BF16, tag="op", bufs=2,
                                    name="vp")
                for j in range(g * 4, g * 4 + 4):
                    fs.append(lambda j=j: vt_tile(j, op_vt))
                return fs

            def heat_thunk():
                def f():
                    hp = op_tile()
                    nc.tensor.matmul(hp[:], winT[:, 0, 0:128],
                                     xtall[:, 0, 0:QC],
                                     start=True, stop=True)
                    nc.scalar.copy(heatout[:], hp[0:1, 0:1])
                return f

            def norm_thunks(qc, pvsb, tail=False):
                thunks = []
                # 1/d = exp(-ln d) on ACT (ln+exp share an act table); both
                # heads' denominator rows in one [1, 2, QC] op pair
                lnt = wkb.tile([65, 2, QC], F32, tag="lnt", bufs=2,
                               name="lnt")
                nc.scalar.activation(lnt[64:65, :, :], pvsb[64:65, :, :], LN)
                rcpb = wkb.tile([65, 2, QC], BF16, tag="rcpb", bufs=2,
                                name="rcpb")
                with nc.allow_low_precision(reason="denom recip"):
                    nc.scalar.activation(rcpb[64:65, :, :], lnt[64:65, :, :],
                                         EXP, scale=-1.0)

                def norm(hh):
                    def f():
                        q0 = qc * QC
                        rb = op_tile()
                        nc.tensor.matmul(rb[0:HD, :], onesf[64:65, 0:HD],
                                         rcpb[64:65, hh, :],
                                         start=True, stop=True)
                        rbs = wkb.tile([HD, QC], F32, tag="rbs", bufs=2,
                                       name="rbs")
                        nc.vector.tensor_copy(rbs[:], rb[0:HD, :])
                        eng = nc.vector if tail else nc.gpsimd
                        with nc.allow_low_precision(reason="ctx bf16"):
                            eng.tensor_mul(
                                ctxt[hh * HD:(hh + 1) * HD, q0:q0 + QC],
                                pvsb[0:64, hh, :], rbs[:])
                    return f

                def oproj(sti):
                    def f():
                        c0 = (qc * 4 + sti) * 128
                        ob = wkb.tile([128, D], BF16, tag="ob", bufs=3,
                                      name="ob")
                        for dc in range(2):
                            op = op_tile()
                            nc.tensor.matmul(op[:], ctxt[:, c0:c0 + 128],
                                             woT[:, dc * QC:(dc + 1) * QC],
                                             start=True, stop=True)
                            eng = nc.scalar if (tail and dc == 0) else nc.vector
                            if eng is nc.scalar:
                                eng.copy(ob[:, dc * QC:(dc + 1) * QC], op[:])
                            else:
                                eng.tensor_copy(ob[:, dc * QC:(dc + 1) * QC],
                                                op[:])
                        nc.sync.dma_start(pout_d[c0:c0 + 128, :], ob[:])
                    return f

                thunks.append(norm(0))
                thunks.append(norm(1))
                for sti in range(4):
                    thunks.append(oproj(sti))
                return thunks

            LAG = 2

            def attention_chunk(qc, fillers, rate):
                q0 = qc * QC
                n_k = 4 * (qc + 1) if causal else NST
                pvs = [psB.tile([65, QC], F32, tag="pv", bufs=2,
                                name=f"pv{hh}") for hh in range(2)]
                window = []

                def emit_pv(pkt, p3, last):
                    js = max(0, pkt - qc * 4) * 128 if causal else 0
                    for hh in range(2):
                        nc.tensor.matmul(
                            pvs[hh][:, js:QC],
                            vnat[:, pkt * 130 + hh * 65:
                                 pkt * 130 + hh * 65 + 65],
                            p3[:, hh, js:QC],
                            start=(pkt == 0), stop=last)

                for kt in range(n_k):
                    st = psB.tile([128, 2, QC], F32, tag="st", bufs=2,
                                  name="st")
                    j = kt - qc * 4
                    js = j * 128 if (causal and j >= 0) else 0
                    for hh in range(2):
                        nc.tensor.matmul(
                            st[:, hh, js:QC],
                            krot[hh * 64:(hh + 1) * 64,
                                 kt * 128:(kt + 1) * 128],
                            qrot[hh * 64:(hh + 1) * 64, q0 + js:q0 + QC],
                            start=True, stop=True)
                    pt = wkb.tile([128, 2, QC], BF16, tag="pt", bufs=4,
                                  name="pt")
                    nc.scalar.activation(pt[:, :, js:QC], st[:, :, js:QC],
                                         EXP, scale=0.125)
                    if causal and j >= 0:
                        nc.vector.tensor_mul(
                            pt[:, :, js:js + 128], pt[:, :, js:js + 128],
                            tri[:])
                    window.append((kt, pt))
                    if len(window) > LAG:
                        emit_pv(*window.pop(0), last=False)
                    for _ in range(rate):
                        if fillers:
                            fillers.pop(0)()
                while fillers:
                    fillers.pop(0)()
                while window:
                    kt_, p_ = window.pop(0)
                    emit_pv(kt_, p_, last=(kt_ == n_k - 1))
                # evict PV accumulators to SBUF, freeing the PSUM banks
                pb = wkb.tile([65, 2, QC], F32, tag="pvsb", bufs=2,
                              name="pvsb")
                for hh in range(2):
                    nc.vector.tensor_copy(pb[:, hh, :], pvs[hh][:])
                return pb

            sb0 = attention_chunk(0, group_fillers(1), rate=2)
            sb1 = attention_chunk(1, group_fillers(2) + norm_thunks(0, sb0),
                                  rate=3)
            sb2 = attention_chunk(2, group_fillers(3) + norm_thunks(1, sb1),
                                  rate=2)
            heat = [heat_thunk() for _ in range(6)]
            f3 = norm_thunks(2, sb2)
            f3 = [f3[0], heat[0], f3[1], heat[1], f3[2], heat[2], f3[3],
                  heat[3], f3[4], heat[4], f3[5], heat[5]]
            sb3 = attention_chunk(3, f3, rate=1)
            for t in norm_thunks(3, sb3, tail=True):
                t()

    _split_multi_waits(nc)
    return nc


_CONSTS = _host_constants()
_PROGRAMS = {}


def _get_program(causal: bool):
    if causal not in _PROGRAMS:
        _PROGRAMS[causal] = _build_program(causal)
    return _PROGRAMS[causal]


def _make_in_maps(x, w_in, w_out):
    x2 = np.asarray(x, dtype=np.float32).reshape(S, D)
    xT = np.ascontiguousarray(x2.T).astype(BF16NP)         # [D, S]
    w_in = np.asarray(w_in, dtype=np.float32)
    w_out = np.asarray(w_out, dtype=np.float32)

    in_maps = []
    for c in range(NCORES):
        r0 = c * EPC
        wq = w_in[r0:r0 + EPC, :]                          # [128, D]
        wk = w_in[D + r0:D + r0 + EPC, :]
        wv = w_in[2 * D + r0:2 * D + r0 + EPC, :]
        winT = np.ascontiguousarray(
            np.concatenate([wq, wk, wv], axis=0).T).astype(BF16NP)  # [D, 384]
        woT = np.ascontiguousarray(w_out[:, r0:r0 + EPC].T).astype(BF16NP)
        in_maps.append({"xT": xT, "winT": winT, "woT": woT,
                        "rotT": _CONSTS[2], "tri": _CONSTS[3],
                        "ident": _CONSTS[4]})
    return in_maps


def kernel(x, w_in, w_out, is_causal):
    causal = bool(np.asarray(is_causal).item())
    nc = _get_program(causal)
    in_maps = _make_in_maps(x, w_in, w_out)
    res = run_bass_kernel_spmd(nc, in_maps, list(range(NCORES)))
    out = np.zeros((S, D), dtype=np.float64)
    for c in range(NCORES):
        out += res.results[c]["pout"].astype(np.float64)
    return out.astype(np.float32).reshape(B, S, D)
